# revision 1
# baseline (speedup 1.0000x reference)
"""Trainium2 Bass kernel for nn_BezierHCPathOptimizer loss.

Math: the reference computes, per sample t,
  T(t)      -- degree-7 Bezier curve in C^8 coefficient space
  speed(t)  = |T'(t)|,  accel(t) = |T''(t)|
  D(t)      = det Sylvester(f_t, f_t')   (f_t monic degree-8 complex poly
              with coefficient vector T(t)) -- this is a polynomial in t of
              degree <= 98 whose roots do NOT depend on the sample points.
  loss = mean(speed * w(log|D|)) + 0.1*sqrt(mean speed^2)
         + 0.01*sqrt(mean accel^2)

So the host factors D(t) = C * prod_i (t - tau_i) once (106-point Chebyshev
interpolation of the 15x15 complex determinant + companion roots, all in
f64), and each NeuronCore evaluates per sample only:
  log|D(t)| = logC' + 0.5 * sum_i ln( ((t-a_i)^2 + b_i^2) * g_i^2 )
(one ScalarE Square + one fused VectorE (add-const)*(mult) per root, with a
log-flush every few roots), two Horner chains for speed^2/accel^2 (shifted
to t-0.5 for f32 conditioning), the softabs weight chain, and row-partial
sums. The 3 scalar sums are all-reduced on the host (8 cores x 128 rows).
"""

import math
import sys

import numpy as np

for _p in ("/root/.axon_site/_ro/trn_rl_repo", "/opt/trn_rl_repo"):
    if _p not in sys.path:
        sys.path.append(_p)

from concourse import bacc, mybir, tile
from concourse.bass_utils import run_bass_kernel_spmd


class _Bacc(bacc.Bacc):
    """Bacc whose activation-table pass sees Exp/Ln/Square only in the
    combined natural_log_exp_and_others table, so the whole kernel runs on
    ONE ACT table load instead of ping-ponging (1.3us per reload). The
    (name, set) list keeps act_info.json order, so emitted ids stay valid;
    every real table does contain Square, we just hide it from the pass."""

    def insert_act_table_loads(self):
        has_activation = any(
            isinstance(i, mybir.InstActivation)
            for b in self.main_func.blocks
            for i in b.instructions
        )
        if not has_activation:
            return
        from concourse.hw_specs import get_activation_tables
        import bass_rust as _bass_rust

        hide = {ACT.Exp, ACT.Ln, ACT.Square}
        tables = []
        for name, s in get_activation_tables(self.m.arch).items():
            if name != "natural_log_exp_and_others":
                s = s - hide
            tables.append((name, s))
        _bass_rust.insert_act_table_loads(self, tables)

F32 = mybir.dt.float32
ALU = mybir.AluOpType
ACT = mybir.ActivationFunctionType

N_CORES = 8
M_SAMPLES = 131072
CHUNK = M_SAMPLES // N_CORES      # 16384
P_DIM = 128
F_DIM = CHUNK // P_DIM            # 128
N_DEG = 8
D_BEZ = 7
FIT_DEG = 98                      # true degree of det Sylvester in t
FIT_NODES = 160                   # overdetermined Chebyshev least-squares fit
FLUSH = 5                         # roots per product before a log flush
FAR_ROOT = 1e4                    # |tau-0.5| beyond which a root's factor ~ const

# engine split of the per-root work (tuned from perfetto traces):
# (n_roots, chain_engine, square_path) -- square_path "act" = ScalarE Square,
# "self" = affine + self-multiply on the chain engine itself.
SPLIT_PLAN = [
    (10, "dve", "act"),
    (10, "dve", "act"),
    (10, "dve", "act"),
    (10, "dve", "act"),
    (10, "dve", "act"),
    (10, "dve", "act"),
    (10, "dve", "act"),
    (10, "dve", "act"),
    (10, "dve", "act"),
    (-1, "dve", "act"),           # -1 = remainder; DVE runs all chains,
]                                 # ScalarE all squares (its idle absorbs them)

DISC_EPS = 1e-12
LEAD_EPS = 1e-12
DELTA_SOFT = 1e-6
EPS_SOFT = 1e-12
ALPHA = 0.1
BETA = 0.01


# ----------------------------------------------------------------------------
# host-side precompute (all f64; control points are tiny)
# ----------------------------------------------------------------------------

def _power_basis(P0, Pd, P_mid):
    """Power-basis coefficients A[j] (j=0..7) of T(t), each (8,2)."""
    P_ctrl = np.concatenate(
        [P0[None], P_mid, Pd[None]], axis=0
    ).astype(np.float64)                       # (8, 8, 2)
    d = D_BEZ
    Mb = np.zeros((d + 1, d + 1))
    for k in range(d + 1):
        for i in range(d - k + 1):
            Mb[k + i, k] += math.comb(d, k) * math.comb(d - k, i) * (-1) ** i
    return np.einsum("jk,knc->jnc", Mb, P_ctrl)  # (8, 8, 2)


def _det_sylvester(Ac, t):
    """det of the reference's 15x15 Sylvester matrix at sample t (complex128).
    Ac: (8 powers, 8 coeffs) complex."""
    n = N_DEG
    c = (Ac * (t ** np.arange(8))[:, None]).sum(0)
    f = np.concatenate([[1.0 + 0j], c])
    g = f[:n] * (n - np.arange(n)).astype(np.complex128)
    s = 2 * n - 1
    S = np.zeros((s, s), np.complex128)
    for i in range(n - 1):
        S[i, i : i + n + 1] = f
    for j in range(n):
        S[n - 1 + j, j : j + n] = g
    return np.linalg.det(S)


def _sq_norm_poly(Amat):
    """coeffs (in t) of sum over components of (poly_c(t))^2."""
    k = Amat.shape[0]
    out = np.zeros(2 * k - 1)
    flat = Amat.reshape(k, -1)
    for c in range(flat.shape[1]):
        out += np.convolve(flat[:, c], flat[:, c])
    return out


def _shift_poly(c, x0):
    """p(t) -> q(u) with q(u) = p(u + x0)."""
    q = np.zeros_like(c)
    for j, cj in enumerate(c):
        for i in range(j + 1):
            q[i] += cj * math.comb(j, i) * x0 ** (j - i)
    return q


def _precompute(P0, Pd, P_mid):
    from numpy.polynomial import chebyshev as _cheb

    A = _power_basis(P0, Pd, P_mid)
    Ac = A[..., 0] + 1j * A[..., 1]

    # --- factor D(t) ---
    deg = FIT_DEG
    nn = FIT_NODES
    nodes = (np.cos(np.pi * (np.arange(nn) + 0.5) / nn) + 1.0) / 2.0
    vals = np.array([_det_sylvester(Ac, t) for t in nodes])
    coef = _cheb.chebfit(2.0 * nodes - 1.0, vals, deg)
    roots = (_cheb.chebroots(coef) + 1.0) / 2.0
    if not np.all(np.isfinite(roots)):
        raise RuntimeError("non-finite roots in discriminant factorization")
    testpt = 0.3781234517  # arbitrary generic point
    logCabs = float(
        np.log(np.abs(_det_sylvester(Ac, testpt)))
        - np.log(np.abs(testpt - roots)).sum()
    )

    # Per-root scale gamma_i = exp(-E_t[ln fac_i]/2) centers each factor's
    # log at 0 over t~U[0,1], so flush-group products stay near 1 -- the
    # ScalarE Ln table is catastrophically wrong below ~1e-18. Far roots
    # (nearly constant factors) are dropped from the device program; their
    # mean-log contribution stays in Lconst either way.
    tg = (np.arange(4096) + 0.5) / 4096.0
    mlog = np.log(
        (tg[None, :] - roots.real[:, None]) ** 2 + roots.imag[:, None] ** 2
    ).mean(1)                                  # E_t[ln fac_i] per root
    Lconst = logCabs + 0.5 * float(mlog.sum())
    keep = np.abs(roots - 0.5) <= FAR_ROOT
    r = roots[keep]
    g = np.exp(-mlog[keep] / 2.0)
    a_g = r.real * g          # ACT Square bias is -a_g, scale is g
    b2g2 = (r.imag * g) ** 2  # stt add-immediate

    # host validation: factored form must reproduce det at random points
    rng = np.random.default_rng(12345)
    tv = rng.random(64)
    direct = np.array([np.log(np.abs(_det_sylvester(Ac, t))) for t in tv])
    fact = Lconst + 0.5 * (
        np.log((tv[:, None] - r.real[None, :]) ** 2 * g[None, :] ** 2
               + (r.imag[None, :] * g[None, :]) ** 2)
    ).sum(1)
    err = np.abs(fact - direct).max()
    if not np.isfinite(err) or err > 0.02:
        raise RuntimeError(f"discriminant factorization validation failed: {err}")

    # --- speed^2 / accel^2 polynomials, shifted to u = t - 0.5 ---
    Ap = A[1:] * np.arange(1, 8)[:, None, None]
    App = Ap[1:] * np.arange(1, 7)[:, None, None]
    sp = _shift_poly(_sq_norm_poly(Ap), 0.5)    # 13 coeffs in u
    ac = _shift_poly(_sq_norm_poly(App), 0.5)   # 11 coeffs in u

    # Deal roots round-robin (sorted by real part) across the planned chains
    # so clustered roots land in different product chains; each chain entry
    # is (engine, square_path, [root indices]).
    order = np.argsort(r.real)
    nch = len(SPLIT_PLAN)
    sizes = []
    left = len(order)
    for cnt, _, _ in SPLIT_PLAN:
        c = left if cnt < 0 else min(cnt, left)
        sizes.append(c)
        left -= c
    caps = sizes[:]
    lists = [[] for _ in range(nch)]
    ci = 0
    for idx in order:
        for _ in range(nch):
            if caps[ci % nch] > 0:
                break
            ci += 1
        lists[ci % nch].append(int(idx))
        caps[ci % nch] -= 1
        ci += 1
    chains = [
        (eng, sqp, lst)
        for (cnt, eng, sqp), lst in zip(SPLIT_PLAN, lists)
    ]

    return dict(
        a_g=a_g, g=g, b2g2=b2g2, chains=chains, Lconst=Lconst, sp=sp, ac=ac
    )


# ----------------------------------------------------------------------------
# device program
# ----------------------------------------------------------------------------

def _logaddexp_const(nc, pool, x, c, out_scale=None, exp_scale=1.0,
                     l_scale=1.0, tagp="", fd=None):
    """logaddexp-ish combine of plane x with constant c via Softplus:
      out_scale*max(x,c) + softplus(-exp_scale*|x - c|)
    (out_scale None means 1). Softplus keeps one ACT table for all three
    logaddexps in the weight chain."""
    w_fd = F_DIM if fd is None else fd
    mx = pool.tile([P_DIM, w_fd], F32, tag=f"mx{tagp}")
    nc.vector.tensor_scalar_max(mx[:], x, float(c))
    mn = pool.tile([P_DIM, w_fd], F32, tag=f"mn{tagp}")
    nc.vector.tensor_scalar_min(mn[:], x, float(c))
    ad = pool.tile([P_DIM, w_fd], F32, tag=f"ad{tagp}")
    nc.vector.tensor_tensor(ad[:], mn[:], mx[:], op=ALU.subtract)
    e = pool.tile([P_DIM, w_fd], F32, tag=f"e{tagp}")
    nc.scalar.activation(
        e[:], ad[:], ACT.Exp, bias=0.0, scale=float(exp_scale)
    )
    l = pool.tile([P_DIM, w_fd], F32, tag=f"l{tagp}")
    nc.scalar.activation(l[:], e[:], ACT.Ln, bias=1.0, scale=1.0)
    out = pool.tile([P_DIM, w_fd], F32, tag=f"lae{tagp}")
    if l_scale != 1.0:
        nc.vector.scalar_tensor_tensor(
            out[:], l[:], float(l_scale), mx[:], op0=ALU.mult, op1=ALU.add
        )
    elif out_scale is None:
        nc.vector.tensor_tensor(out[:], mx[:], l[:], op=ALU.add)
    else:
        nc.vector.scalar_tensor_tensor(
            out[:], mx[:], float(out_scale), l[:], op0=ALU.mult, op1=ALU.add
        )
    return out


def _build_program(consts, debug_planes=()):
    nc = _Bacc(
        "TRN2", target_bir_lowering=False, debug=False, num_devices=N_CORES
    )
    dbg_tiles = {}
    dbg_drams = {}
    for name in debug_planes:
        dbg_drams[name] = nc.dram_tensor(
            f"dbg_{name}", [P_DIM, F_DIM], F32, kind="ExternalOutput"
        )
    ts_in = nc.dram_tensor("ts", [CHUNK], F32, kind="ExternalInput")
    out = nc.dram_tensor("out", [P_DIM, 5], F32, kind="ExternalOutput")

    a_g, g, b2g2 = consts["a_g"], consts["g"], consts["b2g2"]
    chains, Lconst = consts["chains"], consts["Lconst"]
    sp, ac = consts["sp"], consts["ac"]
    nroot = len(a_g)

    # per-partition bias columns for the Square ops (value -a_g[i] each)
    bias_np = np.tile((-a_g).astype(np.float32)[None, :], (P_DIM, 1))
    bias_dram = nc.inline_tensor(np.ascontiguousarray(bias_np), name="sqbias")

    with tile.TileContext(nc) as tc:
        with (
            tc.tile_pool(name="pers", bufs=1) as pers,
            tc.tile_pool(name="sqp", bufs=10) as sqp,
            tc.tile_pool(name="chn", bufs=2) as chn,
        ):
            t = pers.tile([P_DIM, F_DIM], F32, tag="t")
            nc.sync.dma_start(t[:], ts_in.rearrange("(p f) -> p f", p=P_DIM))
            biases = pers.tile([P_DIM, nroot], F32, tag="biases")
            nc.gpsimd.dma_start(biases[:], bias_dram[:])
            partials = pers.tile([P_DIM, 5], F32, tag="partials")

            u = pers.tile([P_DIM, F_DIM], F32, tag="u")
            nc.vector.tensor_scalar_add(u[:], t[:], -0.5)

            # ---- speed^2 chain (Horner in u via fused stt) ----
            def horner(coeffs, xplane, tag):
                z = chn.tile([P_DIM, F_DIM], F32, tag=tag)
                nc.vector.tensor_scalar_mul(z[:], xplane[:], float(coeffs[-1]))
                for cc in coeffs[-2:0:-1]:
                    zn = chn.tile([P_DIM, F_DIM], F32, tag=tag)
                    nc.vector.scalar_tensor_tensor(
                        zn[:], z[:], float(cc), xplane[:],
                        op0=ALU.add, op1=ALU.mult,
                    )
                    z = zn
                return z  # caller adds coeffs[0]

            zsp = horner(sp, u, "zsp")
            sp2 = pers.tile([P_DIM, F_DIM], F32, tag="sp2")
            nc.vector.tensor_scalar(
                sp2[:], zsp[:], float(sp[0]), 0.0, op0=ALU.add, op1=ALU.add,
                accum_out=partials[:, 1:2],
            )
            zac = horner(ac, u, "zac")
            ac2 = pers.tile([P_DIM, F_DIM], F32, tag="ac2")
            nc.vector.tensor_scalar(
                ac2[:], zac[:], float(ac[0]), 0.0, op0=ALU.add, op1=ALU.add,
                accum_out=partials[:, 2:3],
            )

            # ---- discriminant product chains ----
            # Phase 1: all ScalarE Squares up front (one ACT table load);
            # "self"-path squares run on the chain's own engine instead.
            sq_tiles = {}
            for ci, (eng, sqpath, items) in enumerate(chains):
                veng = nc.vector if eng == "dve" else nc.gpsimd
                if sqpath == "act":
                    for idx in items:
                        sq = sqp.tile(
                            [P_DIM, F_DIM], F32, tag="sq", name=f"sq{idx}",
                            bufs=100,
                        )
                        nc.scalar.activation(
                            sq[:], t[:], ACT.Square,
                            bias=biases[:, idx : idx + 1], scale=float(g[idx]),
                        )
                        sq_tiles[idx] = sq
            # Phase 2: product chains on their engines.
            lgs = []
            for ci, (eng, sqpath, items) in enumerate(chains):
                veng = nc.vector if eng == "dve" else nc.gpsimd
                for gstart in range(0, len(items), FLUSH):
                    grp = items[gstart : gstart + FLUSH]
                    P = None
                    for idx in grp:
                        if sqpath == "act":
                            sq = sq_tiles[idx]
                        else:
                            x = sqp.tile(
                                [P_DIM, F_DIM], F32, tag="sqx",
                                name=f"sqx{idx}", bufs=4,
                            )
                            veng.tensor_scalar(
                                x[:], t[:], float(g[idx]), float(a_g[idx]),
                                op0=ALU.mult, op1=ALU.subtract,
                            )
                            sq = sqp.tile(
                                [P_DIM, F_DIM], F32, tag="sq",
                                name=f"sq{idx}", bufs=100,
                            )
                            nc.gpsimd.tensor_tensor(
                                sq[:], x[:], x[:], op=ALU.mult
                            )
                        Pn = chn.tile(
                            [P_DIM, F_DIM], F32, tag=f"P{ci}",
                            name=f"P{ci}_{idx}", bufs=3,
                        )
                        if P is None:
                            veng.tensor_scalar_add(
                                Pn[:], sq[:], float(b2g2[idx])
                            )
                        elif eng == "dve":
                            veng.scalar_tensor_tensor(
                                Pn[:], sq[:], float(b2g2[idx]), P[:],
                                op0=ALU.add, op1=ALU.mult,
                            )
                        else:
                            t1 = chn.tile(
                                [P_DIM, F_DIM], F32, tag=f"T{ci}",
                                name=f"T{ci}_{idx}",
                            )
                            veng.tensor_scalar_add(
                                t1[:], sq[:], float(b2g2[idx])
                            )
                            veng.tensor_tensor(
                                Pn[:], t1[:], P[:], op=ALU.mult
                            )
                        P = Pn
                    lg = chn.tile(
                        [P_DIM, F_DIM], F32, tag="lg", name=f"lg{ci}_{gstart}",
                        bufs=14,
                    )
                    nc.scalar.activation(lg[:], P[:], ACT.Ln, bias=0.0, scale=1.0)
                    lgs.append(lg)
            # serial-sum the group logs on gpsimd; groups finish staggered,
            # so every add except the last overlaps chain compute
            logacc = lgs[0]
            for i, lg in enumerate(lgs[1:]):
                la = chn.tile(
                    [P_DIM, F_DIM], F32, tag="lacc", name=f"lacc{i}", bufs=3,
                )
                nc.gpsimd.tensor_tensor(la[:], logacc[:], lg[:], op=ALU.add)
                logacc = la

            # y = 2*log|det S| = logacc + 2*Lconst; the whole weight chain
            # runs in the doubled-log domain (log1p(1e-12) is below f32
            # resolution everywhere, exactly as in the reference's f32 math).
            y = pers.tile([P_DIM, F_DIM], F32, tag="L")
            nc.vector.tensor_scalar_add(y[:], logacc[:], 2.0 * float(Lconst))

            # Two half-plane streams: stream B's DVE ops overlap stream A's
            # serial Exp/Ln latencies in the otherwise idle kernel tail.
            HF = F_DIM // 2
            w_halves = []
            for hi, h0 in enumerate((0, HF)):
                x1 = _logaddexp_const(
                    nc, chn, y[:, h0 : h0 + HF], 2.0 * math.log(DISC_EPS),
                    exp_scale=0.5, l_scale=2.0, tagp=f"1h{hi}", fd=HF,
                )
                x2 = _logaddexp_const(
                    nc, chn, x1[:], 2.0 * math.log(DELTA_SOFT),
                    tagp=f"2h{hi}", fd=HF,
                )
                # logaddexp(0.5*x2, ln eps_soft) == 0.5*x2 exactly in f32:
                # x2 >= 2 ln(delta) structurally, so eps_soft is < 1 ulp
                # (identically so in the reference's f32 arithmetic).
                wh = pers.tile([P_DIM, HF], F32, tag=f"wh{hi}")
                nc.scalar.activation(
                    wh[:], x2[:], ACT.Exp, bias=0.0, scale=-0.0625
                )
                w_halves.append(wh)
            for _nm, _tl in (("sp2", sp2), ("ac2", ac2), ("logacc", logacc),
                             ("L", y)):
                if _nm in dbg_drams:
                    dbg_tiles[_nm] = _tl

            speed = pers.tile([P_DIM, F_DIM], F32, tag="speed")
            # speed = sqrt(sp2) as exp(0.5*ln(sp2)) -- Ln and Exp live in the
            # same ACT table as Square, so the whole kernel needs ONE
            # activation-table load (Sqrt would force a second).
            lsp = pers.tile([P_DIM, F_DIM], F32, tag="lsp")
            nc.scalar.activation(lsp[:], sp2[:], ACT.Ln, bias=0.0, scale=1.0)
            nc.scalar.activation(speed[:], lsp[:], ACT.Exp, bias=0.0, scale=0.5)
            for hi, h0 in enumerate((0, HF)):
                sw = pers.tile([P_DIM, HF], F32, tag=f"sw{hi}")
                nc.vector.scalar_tensor_tensor(
                    sw[:], speed[:, h0 : h0 + HF], 1.0, w_halves[hi][:],
                    op0=ALU.mult, op1=ALU.mult,
                    accum_out=partials[:, 3 + hi : 4 + hi],
                )

            for name, tl in dbg_tiles.items():
                nc.sync.dma_start(dbg_drams[name][:], tl[:])
            nc.sync.dma_start(out[:], partials[:])

    nc.compile()
    return nc


# ----------------------------------------------------------------------------
# entry point
# ----------------------------------------------------------------------------

_CACHE = {}


def kernel(P0, Pd, P_mid, ts):
    P0 = np.asarray(P0, np.float32)
    Pd = np.asarray(Pd, np.float32)
    P_mid = np.asarray(P_mid, np.float32)
    ts = np.ascontiguousarray(np.asarray(ts, np.float32))
    assert ts.shape == (M_SAMPLES,), ts.shape

    key = (P0.tobytes(), Pd.tobytes(), P_mid.tobytes())
    if key not in _CACHE:
        consts = _precompute(P0, Pd, P_mid)
        _CACHE[key] = (_build_program(consts), consts)
    nc, consts = _CACHE[key]

    in_maps = [
        {"ts": ts[i * CHUNK : (i + 1) * CHUNK]} for i in range(N_CORES)
    ]
    res = run_bass_kernel_spmd(nc, in_maps, list(range(N_CORES)))

    s = np.zeros(5, np.float64)
    for i in range(N_CORES):
        s += res.results[i]["out"].astype(np.float64).sum(0)
    s[0] = s[3] + s[4]
    L_cl = s[0] / M_SAMPLES
    L_d1 = math.sqrt(s[1] / M_SAMPLES)
    L_d2 = math.sqrt(s[2] / M_SAMPLES)
    loss = L_cl + ALPHA * L_d1 + BETA * L_d2
    return np.asarray(loss, dtype=np.float32)

